# revision 1
# baseline (speedup 1.0000x reference)
"""CrossAttention1D Trainium2 kernel.

Problem: B=4, C=1024, L=2048, H=16 heads (D=64). LKV == LQ so the
reference's linear interpolation is the identity and is skipped.

Sharding (8 cores): data-parallel over batch (4) x tensor-parallel over
heads (2 halves of 8 heads). Core c handles batch c//2, heads
(c%2)*8 .. (c%2)*8+8. Each core computes its half of Q/K/V projections
(512 of 1024 channels), attention for its 8 heads, and a partial output
projection Wo[:, shard] @ O (+ residual/bias on even cores). The host
sums the two partials per batch.

Device dataflow per core (all matmuls bf16 with fp32 PSUM accumulate):
  Q  = WqT.T @ query      [512, 2048]  (channels on partitions)
  K  = WkT.T @ context    [512, 2048]
  VT = context.T @ WvT    [2048, 512]  (positions on partitions), stored
       interleaved with a ones column per head -> [2048, 8*65] so the AV
       matmul's 65th row accumulates the softmax denominator for free.
  Per head pair (heads 2t at partitions 0:64, 2t+1 at 64:128 feed
  row-group-paired k=64 matmuls that run concurrently on the PE):
    S^T[j,i] = K_h.T @ Q_h    (per 128-row j-tile, PSUM)
    P^T = exp(S^T / 8)        (ScalarE, PSUM->SBUF bf16)
    O_ext[d(+sum),i] += V_ext^T[jt].T @ P^T[jt]   (m=65, accumulated)
  O = O_ext[0:64] * recip(O_ext[64]) broadcast     -> bf16
  out = WoT.T @ O + resid                          -> fp32 partial
"""

import json

import numpy as np
import ml_dtypes

import concourse.bass as bass
import concourse.mybir as mybir
import concourse.tile as tile
from concourse.bass_utils import run_bass_kernel_spmd

BF16 = ml_dtypes.bfloat16

B, C, L, H, D = 4, 1024, 2048, 16, 64
CS = C // 2          # channel shard per core (512)
HPC = H // 2         # heads per core (8)
NCORES = 8
SCALE = 1.0 / np.sqrt(D)  # 0.125

_DT = mybir.dt

_MAX_WAITS = 1


def _split_drain_waits(nc):
    """Hoist excess per-instruction sync-waits onto preceding NoOps.

    This toolchain's walrus codegen rejects instructions carrying more
    than one sync wait ("Too many sync wait commands"). Hoisting a wait
    onto a NoOp immediately before the instruction on the same engine is
    semantics-preserving (engines execute their stream in order).
    """
    j = json.loads(nc.to_json_bytes())
    n_hoisted = 0
    for fn in j["functions"]:
        for bb in fn["blocks"]:
            out = []
            for inst in bb["instructions"]:
                si = inst.get("sync_info")
                ow = (si or {}).get("on_wait") or []
                if len(ow) > _MAX_WAITS:
                    n_hoisted += 1
                    for i, w in enumerate(ow[: -_MAX_WAITS]):
                        out.append(
                            {
                                "engine": inst["engine"],
                                "ins": [],
                                "outs": [],
                                "name": f"{inst['name']}_hw{i}",
                                "opcode": "NoOp",
                                "debug": inst.get("debug"),
                                "sync_info": {"on_update": [], "on_wait": [w]},
                            }
                        )
                    si["on_wait"] = ow[-_MAX_WAITS:]
                out.append(inst)
            bb["instructions"] = out
    patched = json.dumps(j).encode()
    nc.to_json_bytes = lambda: patched
    return nc


def _build_nc():
    nc = bass.Bass()
    dt = _DT
    bf = dt.bfloat16
    f32 = dt.float32

    q_d = nc.declare_dram_parameter("q_bf", [C, L], bf, isOutput=False)
    ctx_d = nc.declare_dram_parameter("ctx_bf", [C, L], bf, isOutput=False)
    wq_d = nc.declare_dram_parameter("wqT", [C, CS], bf, isOutput=False)
    wk_d = nc.declare_dram_parameter("wkT", [C, CS], bf, isOutput=False)
    wv_d = nc.declare_dram_parameter("wvT", [C, CS], bf, isOutput=False)
    wo_d = nc.declare_dram_parameter("woT", [CS, C], bf, isOutput=False)
    res_d = nc.declare_dram_parameter("resid", [C, L], f32, isOutput=False)
    out_d = nc.declare_dram_parameter("out", [C, L], f32, isOutput=True)

    KT = C // 128        # 8 contraction tiles for projections
    CT = CS // 128       # 4 channel tiles of the shard
    LT = L // 512        # 4 L-tiles of 512
    JT = L // 128        # 16 j-tiles of 128

    with tile.TileContext(nc) as tc:
        with (
            tc.tile_pool(name="const", bufs=1) as cp,
            tc.tile_pool(name="pwork", bufs=3) as pwork,
            tc.tile_pool(name="norm", bufs=2) as normp,
            tc.tile_pool(name="io", bufs=3) as iop,
            tc.tile_pool(name="psum", bufs=3, space="PSUM") as psp,
        ):
            # ---- resident SBUF slabs
            q_sb = cp.tile([128, KT, L], bf)       # query, c_in on partitions
            c_sb = cp.tile([128, KT, L], bf)       # context
            wq_sb = cp.tile([128, KT, CS], bf)
            wk_sb = cp.tile([128, KT, CS], bf)
            wv_sb = cp.tile([128, KT, CS], bf)
            wo_sb = cp.tile([128, CT, C], bf)
            Q_sb = cp.tile([128, CT, L], bf)       # projected Q (bf16)
            K_sb = cp.tile([128, CT, L], bf)
            V_sb = cp.tile([128, JT, HPC * (D + 1)], bf)  # V^T + ones cols
            O_sb = cp.tile([128, CT, L], bf)       # normalized attn output

            qr = q_d.rearrange("(k p) m -> p k m", p=128)
            cr = ctx_d.rearrange("(k p) m -> p k m", p=128)
            wqr = wq_d.rearrange("(k p) m -> p k m", p=128)
            wkr = wk_d.rearrange("(k p) m -> p k m", p=128)
            wvr = wv_d.rearrange("(k p) m -> p k m", p=128)
            wor = wo_d.rearrange("(k p) m -> p k m", p=128)
            for kt in range(KT):
                nc.sync.dma_start(q_sb[:, kt, :], qr[:, kt, :])
                nc.sync.dma_start(c_sb[:, kt, :], cr[:, kt, :])
                nc.sync.dma_start(wq_sb[:, kt, :], wqr[:, kt, :])
                nc.sync.dma_start(wk_sb[:, kt, :], wkr[:, kt, :])
                nc.sync.dma_start(wv_sb[:, kt, :], wvr[:, kt, :])
            for kt in range(CT):
                nc.sync.dma_start(wo_sb[:, kt, :], wor[:, kt, :])

            # ones columns for the AV denominator rows
            v_view = V_sb.rearrange("p j (h e) -> p j h e", e=D + 1)
            for jt in range(JT):
                nc.vector.memset(v_view[:, jt, :, D : D + 1], 1.0)
            ones_sb = cp.tile([1, 64], f32)
            nc.vector.memset(ones_sb, 1.0)

            # ---- projections: Q, K (c_out on partitions), V^T (j on partitions)
            for ct in range(CT):
                for lt in range(LT):
                    ls = slice(lt * 512, (lt + 1) * 512)
                    pq = psp.tile([128, 512], f32, tag="s")
                    for kt in range(KT):
                        nc.tensor.matmul(
                            pq,
                            lhsT=wq_sb[:, kt, ct * 128 : (ct + 1) * 128],
                            rhs=q_sb[:, kt, ls],
                            start=(kt == 0),
                            stop=(kt == KT - 1),
                        )
                    nc.vector.tensor_copy(Q_sb[:, ct, ls], pq)
                    pk = psp.tile([128, 512], f32, tag="s")
                    for kt in range(KT):
                        nc.tensor.matmul(
                            pk,
                            lhsT=wk_sb[:, kt, ct * 128 : (ct + 1) * 128],
                            rhs=c_sb[:, kt, ls],
                            start=(kt == 0),
                            stop=(kt == KT - 1),
                        )
                    nc.vector.tensor_copy(K_sb[:, ct, ls], pk)
            for jt in range(JT):
                pv = psp.tile([128, 512], f32, tag="s")
                for kt in range(KT):
                    nc.tensor.matmul(
                        pv,
                        lhsT=c_sb[:, kt, jt * 128 : (jt + 1) * 128],
                        rhs=wv_sb[:, kt, :],
                        start=(kt == 0),
                        stop=(kt == KT - 1),
                    )
                nc.vector.tensor_copy(
                    v_view[:, jt, :, 0:D],
                    pv.rearrange("p (h d) -> p h d", d=D),
                )

            # ---- attention + output projection, per i-tile epoch.
            # One [128,1024] PSUM tile holds both heads' S^T per j-tile so
            # a single exp covers the pair; the QK->AV pipeline runs 2 j
            # iterations deep, and Wo-projection matmuls for the previous
            # i-tile are interleaved into the loop as PE filler so the PE
            # never idles waiting on ACT (HAM throttle avoidance).
            Exp = mybir.ActivationFunctionType.Exp
            DEPTH = 2

            def emit_wo(mt, it_prev):
                psl = slice(it_prev * 512, (it_prev + 1) * 512)
                po = psp.tile([128, 512], f32, tag="s")
                for kt in range(CT):
                    nc.tensor.matmul(
                        po,
                        lhsT=wo_sb[:, kt, mt * 128 : (mt + 1) * 128],
                        rhs=O_sb[:, kt, psl],
                        start=(kt == 0),
                        stop=(kt == CT - 1),
                    )
                rt = iop.tile([128, 512], f32, tag="res")
                nc.sync.dma_start(rt, res_d[mt * 128 : (mt + 1) * 128, psl])
                ot = iop.tile([128, 512], f32, tag="out")
                nc.vector.tensor_add(ot, po, rt)
                nc.sync.dma_start(out_d[mt * 128 : (mt + 1) * 128, psl], ot)

            for it in range(LT):
                isl = slice(it * 512, (it + 1) * 512)
                wo_queue = list(range(C // 128)) if it > 0 else []
                for tp in range(CT):  # head pair (2*tp, 2*tp+1)
                    pOa = psp.tile([D + 1, 512], f32, tag="acc", bufs=2)
                    pOb = psp.tile([D + 1, 512], f32, tag="acc", bufs=2)
                    pend = []
                    for jt in range(JT + DEPTH):
                        if jt < JT:
                            js = slice(jt * 128, (jt + 1) * 128)
                            pS = psp.tile([128, 1024], f32, tag="s")
                            nc.tensor.matmul(
                                pS[:, 0:512],
                                lhsT=K_sb[0:64, tp, js],
                                rhs=Q_sb[0:64, tp, isl],
                                start=True,
                                stop=True,
                            )
                            nc.tensor.matmul(
                                pS[:, 512:1024],
                                lhsT=K_sb[64:128, tp, js],
                                rhs=Q_sb[64:128, tp, isl],
                                start=True,
                                stop=True,
                            )
                            Pab = pwork.tile([128, 1024], bf, tag="p")
                            nc.scalar.activation(Pab, pS, Exp, scale=SCALE)
                            pend.append((Pab, jt))
                        if len(pend) > (DEPTH if jt < JT else 0):
                            Pab, qjt = pend.pop(0)
                            ha, hb = 2 * tp, 2 * tp + 1
                            nc.tensor.matmul(
                                pOa,
                                lhsT=V_sb[:, qjt, ha * 65 : ha * 65 + 65],
                                rhs=Pab[:, 0:512],
                                start=(qjt == 0),
                                stop=(qjt == JT - 1),
                            )
                            nc.tensor.matmul(
                                pOb,
                                lhsT=V_sb[:, qjt, hb * 65 : hb * 65 + 65],
                                rhs=Pab[:, 512:1024],
                                start=(qjt == 0),
                                stop=(qjt == JT - 1),
                            )
                        if jt % 8 == 7 and wo_queue:
                            emit_wo(wo_queue.pop(0), it - 1)

                    # normalize both heads of the pair
                    for hh, pO in ((2 * tp, pOa), (2 * tp + 1, pOb)):
                        row = normp.tile([1, 512], f32, tag="row")
                        nc.vector.tensor_copy(row, pO[D : D + 1, :])
                        rec = normp.tile([1, 512], f32, tag="rec")
                        nc.vector.reciprocal(rec, row)
                        rb_ps = psp.tile([64, 512], f32, tag="s")
                        nc.tensor.matmul(
                            rb_ps, lhsT=ones_sb, rhs=rec, start=True, stop=True
                        )
                        rbc = normp.tile([64, 512], f32, tag="rbc")
                        nc.vector.tensor_copy(rbc, rb_ps)
                        otmp = normp.tile([64, 512], bf, tag="otmp")
                        nc.vector.tensor_mul(otmp, pO[0:D, :], rbc)
                        poff = (hh % 2) * 64
                        nc.sync.dma_start(
                            O_sb[poff : poff + 64, hh // 2, isl], otmp
                        )
                for mt in wo_queue:
                    emit_wo(mt, it - 1)
            for mt in range(C // 128):
                emit_wo(mt, LT - 1)
    return nc


_NC = None


def _get_nc():
    global _NC
    if _NC is None:
        _NC = _split_drain_waits(_build_nc())
    return _NC


def _make_in_maps(query, context, Wq, Wk, Wv, Wo, bo):
    zeros_res = np.zeros((C, L), np.float32)
    in_maps = []
    for c in range(NCORES):
        b, hf = c // 2, c % 2
        rows = slice(hf * CS, (hf + 1) * CS)
        in_maps.append(
            {
                "q_bf": query[b].astype(BF16),
                "ctx_bf": context[b].astype(BF16),
                "wqT": np.ascontiguousarray(Wq[rows].T).astype(BF16),
                "wkT": np.ascontiguousarray(Wk[rows].T).astype(BF16),
                "wvT": np.ascontiguousarray(Wv[rows].T).astype(BF16),
                "woT": np.ascontiguousarray(Wo[:, rows].T).astype(BF16),
                "resid": (query[b] + bo[:, None]).astype(np.float32)
                if hf == 0
                else zeros_res,
            }
        )
    return in_maps


def _gather(results):
    out = np.empty((B, C, L), np.float32)
    for b in range(B):
        out[b] = results[2 * b]["out"] + results[2 * b + 1]["out"]
    return out


def kernel(query, context, Wq, Wk, Wv, Wo, bo, heads):
    query = np.asarray(query, dtype=np.float32)
    context = np.asarray(context, dtype=np.float32)
    Wq = np.asarray(Wq, dtype=np.float32)
    Wk = np.asarray(Wk, dtype=np.float32)
    Wv = np.asarray(Wv, dtype=np.float32)
    Wo = np.asarray(Wo, dtype=np.float32)
    bo = np.asarray(bo, dtype=np.float32)
    assert int(heads) == H
    assert query.shape == (B, C, L) and context.shape == (B, C, L)

    nc = _get_nc()
    in_maps = _make_in_maps(query, context, Wq, Wk, Wv, Wo, bo)
    res = run_bass_kernel_spmd(nc, in_maps, list(range(NCORES))).results
    return _gather(res)



# revision 21
# speedup vs baseline: 1.3121x; 1.3121x over previous
"""CrossAttention1D Trainium2 kernel.

Problem: B=4, C=1024, L=2048, H=16 heads (D=64). LKV == LQ so the
reference's linear interpolation is the identity and is skipped.

Sharding (8 cores): data-parallel over batch (4) x tensor-parallel over
heads (2 halves of 8 heads). Core c handles batch c//2, heads
(c%2)*8 .. (c%2)*8+8. Each core computes its half of Q/K/V projections
(512 of 1024 channels), attention for its 8 heads, and a partial output
projection Wo[:, shard] @ O (+ residual/bias on even cores). The host
sums the two partials per batch.

Device dataflow per core (all matmuls bf16 with fp32 PSUM accumulate):
  Q  = WqT.T @ query      [512, 2048]  (channels on partitions)
  K  = WkT.T @ context    [512, 2048]
  VT = context.T @ WvT    [2048, 512]  (positions on partitions), stored
       interleaved with a ones column per head -> [2048, 8*65] so the AV
       matmul's 65th row accumulates the softmax denominator for free.
  Attention runs as ONE flattened software pipeline over all 256
  (i-tile, head-pair, j-tile) steps so the ScalarE exp stream never
  stalls at head-pair or i-tile boundaries (ACT is the cadence engine:
  256 x ~1.15us ACTIVATEs):
    S^T[j,i] = K_h.T @ Q_h    (row-group-paired k=64 matmuls, PSUM)
    P^T = exp(S^T / 8)        (ScalarE, PSUM->SBUF bf16)
    O_ext[d(+sum),i] += V_ext^T[jt].T @ P^T[jt]   (m=65, accumulated)
  At each pair's end the raw O_ext is immediately drained PSUM->SBUF
  (bf16 O rows + fp32 denominator row) to release the PSUM accumulator
  banks; softmax normalization (approx-reciprocal + ones-matmul
  partition broadcast + multiply) runs one i-tile later, off the
  critical path, interleaved with the next i-tile's attention. The Wo
  projection matmuls for i-tile t-1 are likewise interleaved as PE
  filler so the PE never idles long enough to drop its HAM clock.
  out = WoT.T @ O + resid(bf16)                   -> fp32 partial
"""

import json

import numpy as np
import ml_dtypes

import concourse.bass as bass
import concourse.mybir as mybir
import concourse.tile as tile
from concourse.bass_utils import run_bass_kernel_spmd

BF16 = ml_dtypes.bfloat16

B, C, L, H, D = 4, 1024, 2048, 16, 64
CS = C // 2          # channel shard per core (512)
HPC = H // 2         # heads per core (8)
NCORES = 8
SCALE = 1.0 / np.sqrt(D)  # 0.125

_DT = mybir.dt

_MAX_WAITS = 1


def _split_drain_waits(nc):
    """Hoist excess per-instruction sync-waits onto preceding NoOps.

    This toolchain's walrus codegen rejects instructions carrying more
    than one sync wait ("Too many sync wait commands"). Hoisting a wait
    onto a NoOp immediately before the instruction on the same engine is
    semantics-preserving (engines execute their stream in order).
    """
    j = json.loads(nc.to_json_bytes())
    n_hoisted = 0
    for fn in j["functions"]:
        for bb in fn["blocks"]:
            out = []
            for inst in bb["instructions"]:
                si = inst.get("sync_info")
                ow = (si or {}).get("on_wait") or []
                if len(ow) > _MAX_WAITS:
                    n_hoisted += 1
                    for i, w in enumerate(ow[: -_MAX_WAITS]):
                        out.append(
                            {
                                "engine": inst["engine"],
                                "ins": [],
                                "outs": [],
                                "name": f"{inst['name']}_hw{i}",
                                "opcode": "NoOp",
                                "debug": inst.get("debug"),
                                "sync_info": {"on_update": [], "on_wait": [w]},
                            }
                        )
                    si["on_wait"] = ow[-_MAX_WAITS:]
                out.append(inst)
            bb["instructions"] = out
    patched = json.dumps(j).encode()
    nc.to_json_bytes = lambda: patched
    return nc


def _build_nc():
    nc = bass.Bass()
    dt = _DT
    bf = dt.bfloat16
    f32 = dt.float32

    q_d = nc.declare_dram_parameter("q_bf", [C, L], bf, isOutput=False)
    ctx_d = nc.declare_dram_parameter("ctx_bf", [C, L], bf, isOutput=False)
    wq_d = nc.declare_dram_parameter("wqT", [C, CS], bf, isOutput=False)
    wk_d = nc.declare_dram_parameter("wkT", [C, CS], bf, isOutput=False)
    wv_d = nc.declare_dram_parameter("wvT", [C, CS], bf, isOutput=False)
    wo_d = nc.declare_dram_parameter("woT", [CS, C], bf, isOutput=False)
    res_d = nc.declare_dram_parameter("resid", [C, L], bf, isOutput=False)
    out_d = nc.declare_dram_parameter("out", [C, L], f32, isOutput=True)

    KT = C // 128        # 8 contraction tiles for projections
    CT = CS // 128       # 4 channel tiles of the shard
    LT = L // 512        # 4 L-tiles of 512
    JT = L // 128        # 16 j-tiles of 128

    with tile.TileContext(nc) as tc:
        with (
            tc.tile_pool(name="const", bufs=1) as cp,
            tc.tile_pool(name="pwork", bufs=3) as pwork,
            tc.tile_pool(name="norm", bufs=2) as normp,
            tc.tile_pool(name="io", bufs=3) as iop,
            tc.tile_pool(name="psum", bufs=3, space="PSUM") as psp,
        ):
            # ---- resident SBUF slabs
            q_sb = cp.tile([128, KT, L], bf)       # query, c_in on partitions
            c_sb = cp.tile([128, KT, L], bf)       # context
            wq_sb = cp.tile([128, KT, CS], bf)
            wk_sb = cp.tile([128, KT, CS], bf)
            wv_sb = cp.tile([128, KT, CS], bf)
            wo_sb = cp.tile([128, CT, C], bf)
            Q_sb = cp.tile([128, CT, L], bf)       # projected Q (bf16)
            K_sb = cp.tile([128, CT, L], bf)
            V_sb = cp.tile([128, JT, HPC * (D + 1)], bf)  # V^T + ones cols
            O_sb = cp.tile([128, CT, L], bf)       # normalized attn output

            qr = q_d.rearrange("(k p) m -> p k m", p=128)
            cr = ctx_d.rearrange("(k p) m -> p k m", p=128)
            wqr = wq_d.rearrange("(k p) m -> p k m", p=128)
            wkr = wk_d.rearrange("(k p) m -> p k m", p=128)
            wvr = wv_d.rearrange("(k p) m -> p k m", p=128)
            wor = wo_d.rearrange("(k p) m -> p k m", p=128)
            for kt in range(KT):
                nc.sync.dma_start(q_sb[:, kt, :], qr[:, kt, :])
                nc.sync.dma_start(c_sb[:, kt, :], cr[:, kt, :])
                nc.sync.dma_start(wq_sb[:, kt, :], wqr[:, kt, :])
                nc.sync.dma_start(wk_sb[:, kt, :], wkr[:, kt, :])
                nc.sync.dma_start(wv_sb[:, kt, :], wvr[:, kt, :])
            for kt in range(CT):
                nc.sync.dma_start(wo_sb[:, kt, :], wor[:, kt, :])

            # ones columns for the AV denominator rows
            v_view = V_sb.rearrange("p j (h e) -> p j h e", e=D + 1)
            for jt in range(JT):
                nc.vector.memset(v_view[:, jt, :, D : D + 1], 1.0)
            ones_sb = cp.tile([1, 64], bf)
            nc.vector.memset(ones_sb, 1.0)
            # denominator staging: head group g's rows sit at the legal
            # partition bases {0,32,64,96} so ONE reciprocal instruction
            # covers 4 heads (DVE reciprocal wall time is 8*freedim
            # cycles regardless of partition count); 2 groups x 2 its
            # in flight. memset so the unused partitions aren't
            # uninitialized reads.
            den_sb = cp.tile([128, 4, 512], f32)
            nc.vector.memset(den_sb, 1.0)

            # ---- projections: Q, K (c_out on partitions), V^T (j on partitions)
            for ct in range(CT):
                for lt in range(LT):
                    ls = slice(lt * 512, (lt + 1) * 512)
                    pq = psp.tile([128, 512], f32, tag="s")
                    for kt in range(KT):
                        nc.tensor.matmul(
                            pq,
                            lhsT=wq_sb[:, kt, ct * 128 : (ct + 1) * 128],
                            rhs=q_sb[:, kt, ls],
                            start=(kt == 0),
                            stop=(kt == KT - 1),
                        )
                    nc.vector.tensor_copy(Q_sb[:, ct, ls], pq)
                    pk = psp.tile([128, 512], f32, tag="s")
                    for kt in range(KT):
                        nc.tensor.matmul(
                            pk,
                            lhsT=wk_sb[:, kt, ct * 128 : (ct + 1) * 128],
                            rhs=c_sb[:, kt, ls],
                            start=(kt == 0),
                            stop=(kt == KT - 1),
                        )
                    nc.vector.tensor_copy(K_sb[:, ct, ls], pk)
            for jt in range(JT):
                pv = psp.tile([128, 512], f32, tag="s")
                for kt in range(KT):
                    nc.tensor.matmul(
                        pv,
                        lhsT=c_sb[:, kt, jt * 128 : (jt + 1) * 128],
                        rhs=wv_sb[:, kt, :],
                        start=(kt == 0),
                        stop=(kt == KT - 1),
                    )
                nc.vector.tensor_copy(
                    v_view[:, jt, :, 0:D],
                    pv.rearrange("p (h d) -> p h d", d=D),
                )

            # ---- attention + output projection, one flattened pipeline.
            # One [128,1024] PSUM tile holds both heads' S^T per j-tile so
            # a single exp covers the pair; the QK->exp->AV pipeline runs
            # DEPTH j-steps deep and spans head-pair/i-tile boundaries.
            Exp = mybir.ActivationFunctionType.Exp
            DEPTH = 2
            NPAIR = CT                     # head pairs per core
            STEPS = LT * NPAIR * JT        # 256 pipeline steps

            def emit_wo(mt, it_prev):
                psl = slice(it_prev * 512, (it_prev + 1) * 512)
                po = psp.tile([128, 512], f32, tag="s")
                for kt in range(CT):
                    nc.tensor.matmul(
                        po,
                        lhsT=wo_sb[:, kt, mt * 128 : (mt + 1) * 128],
                        rhs=O_sb[:, kt, psl],
                        start=(kt == 0),
                        stop=(kt == CT - 1),
                    )
                rt = iop.tile([128, 512], bf, tag="res")
                nc.sync.dma_start(rt, res_d[mt * 128 : (mt + 1) * 128, psl])
                ot = iop.tile([128, 512], f32, tag="out")
                nc.vector.tensor_add(ot, po, rt)
                nc.sync.dma_start(out_d[mt * 128 : (mt + 1) * 128, psl], ot)

            oraw_map = {}

            recf_map = {}

            def emit_norm_head(itp, hh):
                # normalize head hh of i-tile itp's raw attention output
                # (runs one i-tile later, off the AV accumulator critical
                # path, one head per pipeline step to keep PE cadence)
                isl_p = slice(itp * 512, (itp + 1) * 512)
                grp = (itp % 2) * 2 + hh // 4
                if hh % 4 == 0:
                    recf = normp.tile([128, 512], f32, tag="recf", bufs=2)
                    nc.vector.reciprocal(recf, den_sb[:, grp, :])
                    recf_map[grp] = recf
                recf = recf_map[grp]
                pb = 32 * (hh % 4)
                rec = normp.tile([1, 512], bf, tag="rec", bufs=2)
                nc.vector.tensor_copy(rec, recf[pb : pb + 1, :])
                oraw = oraw_map.pop((itp, hh))
                rb_ps = psp.tile([64, 512], f32, tag="s")
                nc.tensor.matmul(
                    rb_ps, lhsT=ones_sb, rhs=rec, start=True, stop=True
                )
                otmp = normp.tile([64, 512], bf, tag="otmp", bufs=2)
                nc.vector.tensor_mul(otmp, oraw, rb_ps)
                poff = (hh % 2) * 64
                nc.sync.dma_start(
                    O_sb[poff : poff + 64, hh // 2, isl_p], otmp
                )

            accs = {}
            pend = []
            wo_queue = []
            for g in range(STEPS + DEPTH):
                if g < STEPS:
                    it, rem = divmod(g, NPAIR * JT)
                    tp, jt = divmod(rem, JT)
                    isl = slice(it * 512, (it + 1) * 512)
                    if rem == 0:
                        wo_queue = list(range(C // 128)) if it > 0 else []
                    if 2 <= rem < 2 + HPC and it > 0:
                        emit_norm_head(it - 1, rem - 2)
                    js = slice(jt * 128, (jt + 1) * 128)
                    pS = psp.tile([128, 1024], f32, tag="s")
                    nc.tensor.matmul(
                        pS[:, 0:512],
                        lhsT=K_sb[0:64, tp, js],
                        rhs=Q_sb[0:64, tp, isl],
                        start=True,
                        stop=True,
                    )
                    nc.tensor.matmul(
                        pS[:, 512:1024],
                        lhsT=K_sb[64:128, tp, js],
                        rhs=Q_sb[64:128, tp, isl],
                        start=True,
                        stop=True,
                    )
                    Pab = pwork.tile([128, 1024], bf, tag="p")
                    nc.scalar.activation(Pab, pS, Exp, scale=SCALE)
                    pend.append((Pab, it, tp, jt))
                if len(pend) > (DEPTH if g < STEPS else 0):
                    Pab, qit, qtp, qjt = pend.pop(0)
                    qisl = slice(qit * 512, (qit + 1) * 512)
                    if qjt == 0:
                        accs[qtp] = (
                            psp.tile(
                                [D + 1, 512], f32, tag="acc", bufs=2,
                                name="pOa",
                            ),
                            psp.tile(
                                [D + 1, 512], f32, tag="acc", bufs=2,
                                name="pOb",
                            ),
                        )
                    pOa, pOb = accs[qtp]
                    ha, hb = 2 * qtp, 2 * qtp + 1
                    nc.tensor.matmul(
                        pOa,
                        lhsT=V_sb[:, qjt, ha * 65 : ha * 65 + 65],
                        rhs=Pab[:, 0:512],
                        start=(qjt == 0),
                        stop=(qjt == JT - 1),
                    )
                    nc.tensor.matmul(
                        pOb,
                        lhsT=V_sb[:, qjt, hb * 65 : hb * 65 + 65],
                        rhs=Pab[:, 512:1024],
                        start=(qjt == 0),
                        stop=(qjt == JT - 1),
                    )
                    if qjt == JT - 1:
                        # drain raw O (bf16) + denominator row to SBUF,
                        # freeing the PSUM accumulator banks for the
                        # next pair
                        for hh, pO in ((ha, pOa), (hb, pOb)):
                            oraw = normp.tile(
                                [D, 512], bf, tag="oraw", bufs=8
                            )
                            nc.vector.tensor_copy(oraw, pO[0:D, :])
                            grp = (qit % 2) * 2 + hh // 4
                            pb = 32 * (hh % 4)
                            nc.vector.tensor_copy(
                                den_sb[pb : pb + 1, grp, :],
                                pO[D : D + 1, :],
                            )
                            oraw_map[(qit, hh)] = oraw
                        del accs[qtp]
                # Wo filler slots sit after the 8 norm slots (rem 2..9) so
                # every head of i-tile it-1 is normalized in O_sb before
                # the first Wo matmul reads it
                grem = g % (NPAIR * JT)
                if (
                    g < STEPS
                    and grem >= 10
                    and (grem - 10) % 6 == 0
                    and wo_queue
                ):
                    emit_wo(wo_queue.pop(0), (g // (NPAIR * JT)) - 1)
            for hh in range(HPC):
                emit_norm_head(LT - 1, hh)
            for mt in wo_queue:
                emit_wo(mt, LT - 2)
            for mt in range(C // 128):
                emit_wo(mt, LT - 1)
    return nc


_NC = None


def _get_nc():
    global _NC
    if _NC is None:
        _NC = _split_drain_waits(_build_nc())
    return _NC


def _make_in_maps(query, context, Wq, Wk, Wv, Wo, bo):
    zeros_res = np.zeros((C, L), BF16)
    in_maps = []
    for c in range(NCORES):
        b, hf = c // 2, c % 2
        rows = slice(hf * CS, (hf + 1) * CS)
        in_maps.append(
            {
                "q_bf": query[b].astype(BF16),
                "ctx_bf": context[b].astype(BF16),
                "wqT": np.ascontiguousarray(Wq[rows].T).astype(BF16),
                "wkT": np.ascontiguousarray(Wk[rows].T).astype(BF16),
                "wvT": np.ascontiguousarray(Wv[rows].T).astype(BF16),
                "woT": np.ascontiguousarray(Wo[:, rows].T).astype(BF16),
                "resid": (query[b] + bo[:, None]).astype(BF16)
                if hf == 0
                else zeros_res,
            }
        )
    return in_maps


def _gather(results):
    out = np.empty((B, C, L), np.float32)
    for b in range(B):
        out[b] = results[2 * b]["out"] + results[2 * b + 1]["out"]
    return out


def kernel(query, context, Wq, Wk, Wv, Wo, bo, heads):
    query = np.asarray(query, dtype=np.float32)
    context = np.asarray(context, dtype=np.float32)
    Wq = np.asarray(Wq, dtype=np.float32)
    Wk = np.asarray(Wk, dtype=np.float32)
    Wv = np.asarray(Wv, dtype=np.float32)
    Wo = np.asarray(Wo, dtype=np.float32)
    bo = np.asarray(bo, dtype=np.float32)
    assert int(heads) == H
    assert query.shape == (B, C, L) and context.shape == (B, C, L)

    nc = _get_nc()
    in_maps = _make_in_maps(query, context, Wq, Wk, Wv, Wo, bo)
    res = run_bass_kernel_spmd(nc, in_maps, list(range(NCORES))).results
    return _gather(res)


# revision 25
# speedup vs baseline: 1.3864x; 1.0566x over previous
"""CrossAttention1D Trainium2 kernel.

Problem: B=4, C=1024, L=2048, H=16 heads (D=64). LKV == LQ so the
reference's linear interpolation is the identity and is skipped.

Sharding (8 cores): data-parallel over batch (4) x tensor-parallel over
heads (2 halves of 8 heads). Core c handles batch c//2, heads
(c%2)*8 .. (c%2)*8+8. Each core computes its half of Q/K/V projections
(512 of 1024 channels), attention for its 8 heads, and a partial output
projection Wo[:, shard] @ O (+ residual/bias on even cores). The host
sums the two partials per batch.

Device dataflow per core (all matmuls bf16 with fp32 PSUM accumulate):
  Q  = WqT.T @ query      [512, 2048]  (channels on partitions)
  K  = WkT.T @ context    [512, 2048]
  VT = context.T @ WvT    [2048, 512]  (positions on partitions), stored
       interleaved with a ones column per head -> [2048, 8*65] so the AV
       matmul's 65th row accumulates the softmax denominator for free.
  Attention runs as ONE flattened software pipeline over all 256
  (i-tile, head-pair, j-tile) steps so the ScalarE exp stream never
  stalls at head-pair or i-tile boundaries (ACT is the cadence engine:
  256 x ~1.15us ACTIVATEs):
    S^T[j,i] = K_h.T @ Q_h    (row-group-paired k=64 matmuls, PSUM)
    P^T = exp(S^T / 8)        (ScalarE, PSUM->SBUF bf16)
    O_ext[d(+sum),i] += V_ext^T[jt].T @ P^T[jt]   (m=65, accumulated)
  At each pair's end the raw O_ext is immediately drained PSUM->SBUF
  (bf16 O rows + fp32 denominator row) to release the PSUM accumulator
  banks; softmax normalization (approx-reciprocal + ones-matmul
  partition broadcast + multiply) runs one i-tile later, off the
  critical path, interleaved with the next i-tile's attention. The Wo
  projection matmuls for i-tile t-1 are likewise interleaved as PE
  filler so the PE never idles long enough to drop its HAM clock.
  out = WoT.T @ O + resid(bf16)                   -> fp32 partial
"""

import json

import numpy as np
import ml_dtypes

import concourse.bass as bass
import concourse.mybir as mybir
import concourse.tile as tile
from concourse.bass_utils import run_bass_kernel_spmd

BF16 = ml_dtypes.bfloat16

B, C, L, H, D = 4, 1024, 2048, 16, 64
CS = C // 2          # channel shard per core (512)
HPC = H // 2         # heads per core (8)
NCORES = 8
SCALE = 1.0 / np.sqrt(D)  # 0.125

_DT = mybir.dt

_MAX_WAITS = 1


def _split_drain_waits(nc):
    """Hoist excess per-instruction sync-waits onto preceding NoOps.

    This toolchain's walrus codegen rejects instructions carrying more
    than one sync wait ("Too many sync wait commands"). Hoisting a wait
    onto a NoOp immediately before the instruction on the same engine is
    semantics-preserving (engines execute their stream in order).
    """
    j = json.loads(nc.to_json_bytes())
    n_hoisted = 0
    for fn in j["functions"]:
        for bb in fn["blocks"]:
            out = []
            for inst in bb["instructions"]:
                si = inst.get("sync_info")
                ow = (si or {}).get("on_wait") or []
                if len(ow) > _MAX_WAITS:
                    n_hoisted += 1
                    for i, w in enumerate(ow[: -_MAX_WAITS]):
                        out.append(
                            {
                                "engine": inst["engine"],
                                "ins": [],
                                "outs": [],
                                "name": f"{inst['name']}_hw{i}",
                                "opcode": "NoOp",
                                "debug": inst.get("debug"),
                                "sync_info": {"on_update": [], "on_wait": [w]},
                            }
                        )
                    si["on_wait"] = ow[-_MAX_WAITS:]
                out.append(inst)
            bb["instructions"] = out
    patched = json.dumps(j).encode()
    nc.to_json_bytes = lambda: patched
    return nc


def _build_nc():
    nc = bass.Bass()
    dt = _DT
    bf = dt.bfloat16
    f32 = dt.float32

    q_d = nc.declare_dram_parameter("q_bf", [C, L], bf, isOutput=False)
    ctx_d = nc.declare_dram_parameter("ctx_bf", [C, L], bf, isOutput=False)
    wq_d = nc.declare_dram_parameter("wqT", [C, CS], bf, isOutput=False)
    wk_d = nc.declare_dram_parameter("wkT", [C, CS], bf, isOutput=False)
    wv_d = nc.declare_dram_parameter("wvT", [C, CS], bf, isOutput=False)
    wo_d = nc.declare_dram_parameter("woT", [CS, C], bf, isOutput=False)
    res_d = nc.declare_dram_parameter("resid", [C, L], bf, isOutput=False)
    out_d = nc.declare_dram_parameter("out", [C, L], f32, isOutput=True)

    KT = C // 128        # 8 contraction tiles for projections
    CT = CS // 128       # 4 channel tiles of the shard
    LT = L // 512        # 4 L-tiles of 512
    JT = L // 128        # 16 j-tiles of 128

    with tile.TileContext(nc) as tc:
        with (
            tc.tile_pool(name="const", bufs=1) as cp,
            tc.tile_pool(name="pwork", bufs=3) as pwork,
            tc.tile_pool(name="norm", bufs=2) as normp,
            tc.tile_pool(name="io", bufs=3) as iop,
            tc.tile_pool(name="psum", bufs=3, space="PSUM") as psp,
        ):
            # ---- resident SBUF slabs
            q_sb = cp.tile([128, KT, L], bf)       # query, c_in on partitions
            c_sb = cp.tile([128, KT, L], bf)       # context
            wq_sb = cp.tile([128, KT, CS], bf)
            wk_sb = cp.tile([128, KT, CS], bf)
            wv_sb = cp.tile([128, KT, CS], bf)
            wo_sb = cp.tile([128, CT, C], bf)
            Q_sb = cp.tile([128, CT, L], bf)       # projected Q (bf16)
            K_sb = cp.tile([128, CT, L], bf)
            V_sb = cp.tile([128, JT, HPC * (D + 1)], bf)  # V^T + ones cols
            O_sb = cp.tile([128, CT, L], bf)       # normalized attn output

            qr = q_d.rearrange("(k p) m -> p k m", p=128)
            cr = ctx_d.rearrange("(k p) m -> p k m", p=128)
            wqr = wq_d.rearrange("(k p) m -> p k m", p=128)
            wkr = wk_d.rearrange("(k p) m -> p k m", p=128)
            wvr = wv_d.rearrange("(k p) m -> p k m", p=128)
            wor = wo_d.rearrange("(k p) m -> p k m", p=128)
            # context + K/V weights first: the K projection (first PE
            # work) only needs these, so it starts ~10us earlier
            for kt in range(KT):
                nc.sync.dma_start(c_sb[:, kt, :], cr[:, kt, :])
                nc.sync.dma_start(wk_sb[:, kt, :], wkr[:, kt, :])
                nc.sync.dma_start(wv_sb[:, kt, :], wvr[:, kt, :])
            for kt in range(KT):
                nc.sync.dma_start(q_sb[:, kt, :], qr[:, kt, :])
                nc.sync.dma_start(wq_sb[:, kt, :], wqr[:, kt, :])
            for kt in range(CT):
                nc.sync.dma_start(wo_sb[:, kt, :], wor[:, kt, :])

            # ones columns for the AV denominator rows
            v_view = V_sb.rearrange("p j (h e) -> p j h e", e=D + 1)
            for jt in range(JT):
                nc.vector.memset(v_view[:, jt, :, D : D + 1], 1.0)
            ones_sb = cp.tile([1, 64], bf)
            nc.vector.memset(ones_sb, 1.0)
            # denominator staging: head group g's rows sit at the legal
            # partition bases {0,32,64,96} so ONE reciprocal instruction
            # covers 4 heads (DVE reciprocal wall time is 8*freedim
            # cycles regardless of partition count); 2 groups x 2 its
            # in flight. memset so the unused partitions aren't
            # uninitialized reads.
            den_sb = cp.tile([128, 4, 512], f32)
            nc.vector.memset(den_sb, 1.0)

            # ---- projections: Q, K (c_out on partitions), V^T (j on partitions)
            def emit_qproj(ct, lt):
                ls = slice(lt * 512, (lt + 1) * 512)
                pq = psp.tile([128, 512], f32, tag="s")
                for kt in range(KT):
                    nc.tensor.matmul(
                        pq,
                        lhsT=wq_sb[:, kt, ct * 128 : (ct + 1) * 128],
                        rhs=q_sb[:, kt, ls],
                        start=(kt == 0),
                        stop=(kt == KT - 1),
                    )
                nc.vector.tensor_copy(Q_sb[:, ct, ls], pq)

            for ct in range(CT):
                for lt in range(LT):
                    ls = slice(lt * 512, (lt + 1) * 512)
                    pk = psp.tile([128, 512], f32, tag="s")
                    for kt in range(KT):
                        nc.tensor.matmul(
                            pk,
                            lhsT=wk_sb[:, kt, ct * 128 : (ct + 1) * 128],
                            rhs=c_sb[:, kt, ls],
                            start=(kt == 0),
                            stop=(kt == KT - 1),
                        )
                    nc.vector.tensor_copy(K_sb[:, ct, ls], pk)
            for jt in range(JT):
                pv = psp.tile([128, 512], f32, tag="s")
                for kt in range(KT):
                    nc.tensor.matmul(
                        pv,
                        lhsT=c_sb[:, kt, jt * 128 : (jt + 1) * 128],
                        rhs=wv_sb[:, kt, :],
                        start=(kt == 0),
                        stop=(kt == KT - 1),
                    )
                nc.vector.tensor_copy(
                    v_view[:, jt, :, 0:D],
                    pv.rearrange("p (h d) -> p h d", d=D),
                )
            # only i-tile 0's Q chunks up front; the rest are emitted as
            # PE filler during attention (chunk (ct, it+1) during it)
            for ct in range(CT):
                emit_qproj(ct, 0)

            # ---- attention + output projection, one flattened pipeline.
            # One [128,1024] PSUM tile holds both heads' S^T per j-tile so
            # a single exp covers the pair; the QK->exp->AV pipeline runs
            # DEPTH j-steps deep and spans head-pair/i-tile boundaries.
            Exp = mybir.ActivationFunctionType.Exp
            DEPTH = 2
            NPAIR = CT                     # head pairs per core
            STEPS = LT * NPAIR * JT        # 256 pipeline steps

            def emit_wo(mt, it_prev):
                psl = slice(it_prev * 512, (it_prev + 1) * 512)
                po = psp.tile([128, 512], f32, tag="s")
                for kt in range(CT):
                    nc.tensor.matmul(
                        po,
                        lhsT=wo_sb[:, kt, mt * 128 : (mt + 1) * 128],
                        rhs=O_sb[:, kt, psl],
                        start=(kt == 0),
                        stop=(kt == CT - 1),
                    )
                rt = iop.tile([128, 512], bf, tag="res")
                nc.sync.dma_start(rt, res_d[mt * 128 : (mt + 1) * 128, psl])
                ot = iop.tile([128, 512], f32, tag="out")
                nc.vector.tensor_add(ot, po, rt)
                nc.sync.dma_start(out_d[mt * 128 : (mt + 1) * 128, psl], ot)

            oraw_map = {}

            recf_map = {}

            def emit_norm_head(itp, hh):
                # normalize head hh of i-tile itp's raw attention output
                # (runs one i-tile later, off the AV accumulator critical
                # path, one head per pipeline step to keep PE cadence)
                isl_p = slice(itp * 512, (itp + 1) * 512)
                grp = (itp % 2) * 2 + hh // 4
                if hh % 4 == 0:
                    recf = normp.tile([128, 512], f32, tag="recf", bufs=2)
                    nc.vector.reciprocal(recf, den_sb[:, grp, :])
                    recf_map[grp] = recf
                recf = recf_map[grp]
                pb = 32 * (hh % 4)
                rec = normp.tile([1, 512], bf, tag="rec", bufs=2)
                nc.vector.tensor_copy(rec, recf[pb : pb + 1, :])
                oraw = oraw_map.pop((itp, hh))
                rb_ps = psp.tile([64, 512], f32, tag="s")
                nc.tensor.matmul(
                    rb_ps, lhsT=ones_sb, rhs=rec, start=True, stop=True
                )
                otmp = normp.tile([64, 512], bf, tag="otmp", bufs=2)
                nc.vector.tensor_mul(otmp, oraw, rb_ps)
                poff = (hh % 2) * 64
                nc.sync.dma_start(
                    O_sb[poff : poff + 64, hh // 2, isl_p], otmp
                )

            accs = {}
            pend = []
            wo_queue = []
            for g in range(STEPS + DEPTH):
                if g < STEPS:
                    it, rem = divmod(g, NPAIR * JT)
                    tp, jt = divmod(rem, JT)
                    isl = slice(it * 512, (it + 1) * 512)
                    if rem == 0:
                        wo_queue = list(range(C // 128)) if it > 0 else []
                    if 2 <= rem < 2 + HPC and it > 0:
                        emit_norm_head(it - 1, rem - 2)
                    # deferred Q projection: chunk (tp', it+1) as filler
                    if it < LT - 1 and rem >= 13 and (rem - 13) % 14 == 0:
                        emit_qproj((rem - 13) // 14, it + 1)
                    # last i-tile: heads 0-3 normalize as soon as their
                    # pairs drain, shortening the epilogue
                    if it == LT - 1 and 52 <= rem < 56:
                        emit_norm_head(LT - 1, rem - 52)
                    js = slice(jt * 128, (jt + 1) * 128)
                    pS = psp.tile([128, 1024], f32, tag="s")
                    nc.tensor.matmul(
                        pS[:, 0:512],
                        lhsT=K_sb[0:64, tp, js],
                        rhs=Q_sb[0:64, tp, isl],
                        start=True,
                        stop=True,
                    )
                    nc.tensor.matmul(
                        pS[:, 512:1024],
                        lhsT=K_sb[64:128, tp, js],
                        rhs=Q_sb[64:128, tp, isl],
                        start=True,
                        stop=True,
                    )
                    Pab = pwork.tile([128, 1024], bf, tag="p")
                    nc.scalar.activation(Pab, pS, Exp, scale=SCALE)
                    pend.append((Pab, it, tp, jt))
                if len(pend) > (DEPTH if g < STEPS else 0):
                    Pab, qit, qtp, qjt = pend.pop(0)
                    qisl = slice(qit * 512, (qit + 1) * 512)
                    if qjt == 0:
                        accs[qtp] = (
                            psp.tile(
                                [D + 1, 512], f32, tag="acc", bufs=2,
                                name="pOa",
                            ),
                            psp.tile(
                                [D + 1, 512], f32, tag="acc", bufs=2,
                                name="pOb",
                            ),
                        )
                    pOa, pOb = accs[qtp]
                    ha, hb = 2 * qtp, 2 * qtp + 1
                    nc.tensor.matmul(
                        pOa,
                        lhsT=V_sb[:, qjt, ha * 65 : ha * 65 + 65],
                        rhs=Pab[:, 0:512],
                        start=(qjt == 0),
                        stop=(qjt == JT - 1),
                    )
                    nc.tensor.matmul(
                        pOb,
                        lhsT=V_sb[:, qjt, hb * 65 : hb * 65 + 65],
                        rhs=Pab[:, 512:1024],
                        start=(qjt == 0),
                        stop=(qjt == JT - 1),
                    )
                    if qjt == JT - 1:
                        # drain raw O (bf16) + denominator row to SBUF,
                        # freeing the PSUM accumulator banks for the
                        # next pair
                        for hh, pO in ((ha, pOa), (hb, pOb)):
                            oraw = normp.tile(
                                [D, 512], bf, tag="oraw", bufs=8
                            )
                            nc.vector.tensor_copy(oraw, pO[0:D, :])
                            grp = (qit % 2) * 2 + hh // 4
                            pb = 32 * (hh % 4)
                            nc.vector.tensor_copy(
                                den_sb[pb : pb + 1, grp, :],
                                pO[D : D + 1, :],
                            )
                            oraw_map[(qit, hh)] = oraw
                        del accs[qtp]
                # Wo filler slots sit after the 8 norm slots (rem 2..9) so
                # every head of i-tile it-1 is normalized in O_sb before
                # the first Wo matmul reads it
                grem = g % (NPAIR * JT)
                if (
                    g < STEPS
                    and grem >= 10
                    and (grem - 10) % 6 == 0
                    and wo_queue
                ):
                    emit_wo(wo_queue.pop(0), (g // (NPAIR * JT)) - 1)
            for hh in range(4, HPC):
                emit_norm_head(LT - 1, hh)
            for mt in wo_queue:
                emit_wo(mt, LT - 2)
            for mt in range(C // 128):
                emit_wo(mt, LT - 1)
    return nc


_NC = None


def _get_nc():
    global _NC
    if _NC is None:
        _NC = _split_drain_waits(_build_nc())
    return _NC


def _make_in_maps(query, context, Wq, Wk, Wv, Wo, bo):
    zeros_res = np.zeros((C, L), BF16)
    in_maps = []
    for c in range(NCORES):
        b, hf = c // 2, c % 2
        rows = slice(hf * CS, (hf + 1) * CS)
        in_maps.append(
            {
                "q_bf": query[b].astype(BF16),
                "ctx_bf": context[b].astype(BF16),
                "wqT": np.ascontiguousarray(Wq[rows].T).astype(BF16),
                "wkT": np.ascontiguousarray(Wk[rows].T).astype(BF16),
                "wvT": np.ascontiguousarray(Wv[rows].T).astype(BF16),
                "woT": np.ascontiguousarray(Wo[:, rows].T).astype(BF16),
                "resid": (query[b] + bo[:, None]).astype(BF16)
                if hf == 0
                else zeros_res,
            }
        )
    return in_maps


def _gather(results):
    out = np.empty((B, C, L), np.float32)
    for b in range(B):
        out[b] = results[2 * b]["out"] + results[2 * b + 1]["out"]
    return out


def kernel(query, context, Wq, Wk, Wv, Wo, bo, heads):
    query = np.asarray(query, dtype=np.float32)
    context = np.asarray(context, dtype=np.float32)
    Wq = np.asarray(Wq, dtype=np.float32)
    Wk = np.asarray(Wk, dtype=np.float32)
    Wv = np.asarray(Wv, dtype=np.float32)
    Wo = np.asarray(Wo, dtype=np.float32)
    bo = np.asarray(bo, dtype=np.float32)
    assert int(heads) == H
    assert query.shape == (B, C, L) and context.shape == (B, C, L)

    nc = _get_nc()
    in_maps = _make_in_maps(query, context, Wq, Wk, Wv, Wo, bo)
    res = run_bass_kernel_spmd(nc, in_maps, list(range(NCORES))).results
    return _gather(res)


# revision 28
# speedup vs baseline: 1.3966x; 1.0074x over previous
"""CrossAttention1D Trainium2 kernel.

Problem: B=4, C=1024, L=2048, H=16 heads (D=64). LKV == LQ so the
reference's linear interpolation is the identity and is skipped.

Sharding (8 cores): data-parallel over batch (4) x tensor-parallel over
heads (2 halves of 8 heads). Core c handles batch c//2, heads
(c%2)*8 .. (c%2)*8+8. Each core computes its half of Q/K/V projections
(512 of 1024 channels), attention for its 8 heads, and a partial output
projection Wo[:, shard] @ O (+ residual/bias on even cores). The host
sums the two partials per batch.

Device dataflow per core (all matmuls bf16 with fp32 PSUM accumulate):
  Q  = WqT.T @ query      [512, 2048]  (channels on partitions)
  K  = WkT.T @ context    [512, 2048]
  VT = context.T @ WvT    [2048, 512]  (positions on partitions), stored
       interleaved with a ones column per head -> [2048, 8*65] so the AV
       matmul's 65th row accumulates the softmax denominator for free.
  Attention runs as ONE flattened software pipeline over all 256
  (i-tile, head-pair, j-tile) steps so the ScalarE exp stream never
  stalls at head-pair or i-tile boundaries (ACT is the cadence engine:
  256 x ~1.15us ACTIVATEs):
    S^T[j,i] = K_h.T @ Q_h    (row-group-paired k=64 matmuls, PSUM)
    P^T = exp(S^T / 8)        (ScalarE, PSUM->SBUF bf16)
    O_ext[d(+sum),i] += V_ext^T[jt].T @ P^T[jt]   (m=65, accumulated)
  At each pair's end the raw O_ext is immediately drained PSUM->SBUF
  (bf16 O rows + fp32 denominator row) to release the PSUM accumulator
  banks; softmax normalization (approx-reciprocal + ones-matmul
  partition broadcast + multiply) runs one i-tile later, off the
  critical path, interleaved with the next i-tile's attention. The Wo
  projection matmuls for i-tile t-1 are likewise interleaved as PE
  filler so the PE never idles long enough to drop its HAM clock.
  out = WoT.T @ O + resid(bf16)                   -> fp32 partial
"""

import json

import numpy as np
import ml_dtypes

import concourse.bass as bass
import concourse.mybir as mybir
import concourse.tile as tile
from concourse.bass_utils import run_bass_kernel_spmd

BF16 = ml_dtypes.bfloat16

B, C, L, H, D = 4, 1024, 2048, 16, 64
CS = C // 2          # channel shard per core (512)
HPC = H // 2         # heads per core (8)
NCORES = 8
SCALE = 1.0 / np.sqrt(D)  # 0.125

_DT = mybir.dt

_MAX_WAITS = 1


def _split_drain_waits(nc):
    """Hoist excess per-instruction sync-waits onto preceding NoOps.

    This toolchain's walrus codegen rejects instructions carrying more
    than one sync wait ("Too many sync wait commands"). Hoisting a wait
    onto a NoOp immediately before the instruction on the same engine is
    semantics-preserving (engines execute their stream in order).
    """
    j = json.loads(nc.to_json_bytes())
    n_hoisted = 0
    for fn in j["functions"]:
        for bb in fn["blocks"]:
            out = []
            for inst in bb["instructions"]:
                si = inst.get("sync_info")
                ow = (si or {}).get("on_wait") or []
                if len(ow) > _MAX_WAITS:
                    n_hoisted += 1
                    for i, w in enumerate(ow[: -_MAX_WAITS]):
                        out.append(
                            {
                                "engine": inst["engine"],
                                "ins": [],
                                "outs": [],
                                "name": f"{inst['name']}_hw{i}",
                                "opcode": "NoOp",
                                "debug": inst.get("debug"),
                                "sync_info": {"on_update": [], "on_wait": [w]},
                            }
                        )
                    si["on_wait"] = ow[-_MAX_WAITS:]
                out.append(inst)
            bb["instructions"] = out
    patched = json.dumps(j).encode()
    nc.to_json_bytes = lambda: patched
    return nc


def _build_nc():
    nc = bass.Bass()
    dt = _DT
    bf = dt.bfloat16
    f32 = dt.float32

    q_d = nc.declare_dram_parameter("q_bf", [C, L], bf, isOutput=False)
    ctx_d = nc.declare_dram_parameter("ctx_bf", [C, L], bf, isOutput=False)
    wq_d = nc.declare_dram_parameter("wqT", [C, CS], bf, isOutput=False)
    wk_d = nc.declare_dram_parameter("wkT", [C, CS], bf, isOutput=False)
    wv_d = nc.declare_dram_parameter("wvT", [C, CS], bf, isOutput=False)
    wo_d = nc.declare_dram_parameter("woT", [CS, C], bf, isOutput=False)
    res_d = nc.declare_dram_parameter("resid", [C, L], bf, isOutput=False)
    out_d = nc.declare_dram_parameter("out", [C, L], bf, isOutput=True)

    KT = C // 128        # 8 contraction tiles for projections
    CT = CS // 128       # 4 channel tiles of the shard
    LT = L // 512        # 4 L-tiles of 512
    JT = L // 128        # 16 j-tiles of 128

    with tile.TileContext(nc) as tc:
        with (
            tc.tile_pool(name="const", bufs=1) as cp,
            tc.tile_pool(name="pwork", bufs=3) as pwork,
            tc.tile_pool(name="norm", bufs=2) as normp,
            tc.tile_pool(name="io", bufs=3) as iop,
            tc.tile_pool(name="psum", bufs=3, space="PSUM") as psp,
        ):
            # ---- resident SBUF slabs
            q_sb = cp.tile([128, KT, L], bf)       # query, c_in on partitions
            c_sb = cp.tile([128, KT, L], bf)       # context
            wq_sb = cp.tile([128, KT, CS], bf)
            wk_sb = cp.tile([128, KT, CS], bf)
            wv_sb = cp.tile([128, KT, CS], bf)
            wo_sb = cp.tile([128, CT, C], bf)
            Q_sb = cp.tile([128, CT, L], bf)       # projected Q (bf16)
            K_sb = cp.tile([128, CT, L], bf)
            V_sb = cp.tile([128, JT, HPC * (D + 1)], bf)  # V^T + ones cols
            O_sb = cp.tile([128, CT, L], bf)       # normalized attn output

            qr = q_d.rearrange("(k p) m -> p k m", p=128)
            cr = ctx_d.rearrange("(k p) m -> p k m", p=128)
            wqr = wq_d.rearrange("(k p) m -> p k m", p=128)
            wkr = wk_d.rearrange("(k p) m -> p k m", p=128)
            wvr = wv_d.rearrange("(k p) m -> p k m", p=128)
            wor = wo_d.rearrange("(k p) m -> p k m", p=128)
            # context + K/V weights first: the K projection (first PE
            # work) only needs these, so it starts ~10us earlier
            for kt in range(KT):
                nc.sync.dma_start(c_sb[:, kt, :], cr[:, kt, :])
                nc.sync.dma_start(wk_sb[:, kt, :], wkr[:, kt, :])
                nc.sync.dma_start(wv_sb[:, kt, :], wvr[:, kt, :])
            for kt in range(KT):
                nc.sync.dma_start(q_sb[:, kt, :], qr[:, kt, :])
                nc.sync.dma_start(wq_sb[:, kt, :], wqr[:, kt, :])
            for kt in range(CT):
                nc.sync.dma_start(wo_sb[:, kt, :], wor[:, kt, :])

            # ones columns for the AV denominator rows
            v_view = V_sb.rearrange("p j (h e) -> p j h e", e=D + 1)
            for jt in range(JT):
                nc.vector.memset(v_view[:, jt, :, D : D + 1], 1.0)
            ones_sb = cp.tile([1, 64], bf)
            nc.vector.memset(ones_sb, 1.0)
            # denominator staging: head group g's rows sit at the legal
            # partition bases {0,32,64,96} so ONE reciprocal instruction
            # covers 4 heads (DVE reciprocal wall time is 8*freedim
            # cycles regardless of partition count); 2 groups x 2 its
            # in flight. memset so the unused partitions aren't
            # uninitialized reads.
            den_sb = cp.tile([128, 4, 512], f32)
            nc.vector.memset(den_sb, 1.0)

            # ---- projections: Q, K (c_out on partitions), V^T (j on partitions)
            def emit_qproj(ct, lt):
                ls = slice(lt * 512, (lt + 1) * 512)
                pq = psp.tile([128, 512], f32, tag="s")
                for kt in range(KT):
                    nc.tensor.matmul(
                        pq,
                        lhsT=wq_sb[:, kt, ct * 128 : (ct + 1) * 128],
                        rhs=q_sb[:, kt, ls],
                        start=(kt == 0),
                        stop=(kt == KT - 1),
                    )
                nc.vector.tensor_copy(Q_sb[:, ct, ls], pq)

            for ct in range(CT):
                for lt in range(LT):
                    ls = slice(lt * 512, (lt + 1) * 512)
                    pk = psp.tile([128, 512], f32, tag="s")
                    for kt in range(KT):
                        nc.tensor.matmul(
                            pk,
                            lhsT=wk_sb[:, kt, ct * 128 : (ct + 1) * 128],
                            rhs=c_sb[:, kt, ls],
                            start=(kt == 0),
                            stop=(kt == KT - 1),
                        )
                    nc.vector.tensor_copy(K_sb[:, ct, ls], pk)
            for jt in range(JT):
                pv = psp.tile([128, 512], f32, tag="s")
                for kt in range(KT):
                    nc.tensor.matmul(
                        pv,
                        lhsT=c_sb[:, kt, jt * 128 : (jt + 1) * 128],
                        rhs=wv_sb[:, kt, :],
                        start=(kt == 0),
                        stop=(kt == KT - 1),
                    )
                nc.vector.tensor_copy(
                    v_view[:, jt, :, 0:D],
                    pv.rearrange("p (h d) -> p h d", d=D),
                )
            # only i-tile 0's Q chunks up front; the rest are emitted as
            # PE filler during attention (chunk (ct, it+1) during it)
            for ct in range(CT):
                emit_qproj(ct, 0)

            # ---- attention + output projection, one flattened pipeline.
            # One [128,1024] PSUM tile holds both heads' S^T per j-tile so
            # a single exp covers the pair; the QK->exp->AV pipeline runs
            # DEPTH j-steps deep and spans head-pair/i-tile boundaries.
            Exp = mybir.ActivationFunctionType.Exp
            DEPTH = 2
            NPAIR = CT                     # head pairs per core
            STEPS = LT * NPAIR * JT        # 256 pipeline steps

            def emit_wo(mt, it_prev):
                psl = slice(it_prev * 512, (it_prev + 1) * 512)
                po = psp.tile([128, 512], f32, tag="s")
                for kt in range(CT):
                    nc.tensor.matmul(
                        po,
                        lhsT=wo_sb[:, kt, mt * 128 : (mt + 1) * 128],
                        rhs=O_sb[:, kt, psl],
                        start=(kt == 0),
                        stop=(kt == CT - 1),
                    )
                rt = iop.tile([128, 512], bf, tag="res")
                nc.sync.dma_start(rt, res_d[mt * 128 : (mt + 1) * 128, psl])
                ot = iop.tile([128, 512], bf, tag="out")
                nc.vector.tensor_add(ot, po, rt)
                nc.sync.dma_start(out_d[mt * 128 : (mt + 1) * 128, psl], ot)

            oraw_map = {}

            recf_map = {}

            def emit_norm_head(itp, hh):
                # normalize head hh of i-tile itp's raw attention output
                # (runs one i-tile later, off the AV accumulator critical
                # path, one head per pipeline step to keep PE cadence)
                isl_p = slice(itp * 512, (itp + 1) * 512)
                grp = (itp % 2) * 2 + hh // 4
                if hh % 4 == 0:
                    recf = normp.tile([128, 512], f32, tag="recf", bufs=2)
                    nc.vector.reciprocal(recf, den_sb[:, grp, :])
                    recf_map[grp] = recf
                recf = recf_map[grp]
                pb = 32 * (hh % 4)
                rec = normp.tile([1, 512], bf, tag="rec", bufs=2)
                nc.vector.tensor_copy(rec, recf[pb : pb + 1, :])
                oraw = oraw_map.pop((itp, hh))
                rb_ps = psp.tile([64, 512], f32, tag="s")
                nc.tensor.matmul(
                    rb_ps, lhsT=ones_sb, rhs=rec, start=True, stop=True
                )
                otmp = normp.tile([64, 512], bf, tag="otmp", bufs=2)
                nc.vector.tensor_mul(otmp, oraw, rb_ps)
                poff = (hh % 2) * 64
                nc.sync.dma_start(
                    O_sb[poff : poff + 64, hh // 2, isl_p], otmp
                )

            accs = {}
            pend = []
            wo_queue = []
            for g in range(STEPS + DEPTH):
                if g < STEPS:
                    it, rem = divmod(g, NPAIR * JT)
                    tp, jt = divmod(rem, JT)
                    isl = slice(it * 512, (it + 1) * 512)
                    if rem == 0:
                        wo_queue = list(range(C // 128)) if it > 0 else []
                    if 2 <= rem < 2 + HPC and it > 0:
                        emit_norm_head(it - 1, rem - 2)
                    # deferred Q projection: chunk (tp', it+1) as filler
                    if it < LT - 1 and rem >= 13 and (rem - 13) % 14 == 0:
                        emit_qproj((rem - 13) // 14, it + 1)
                    # last i-tile: heads 0-3 normalize as soon as their
                    # pairs drain, shortening the epilogue
                    if it == LT - 1 and 52 <= rem < 56:
                        emit_norm_head(LT - 1, rem - 52)
                    js = slice(jt * 128, (jt + 1) * 128)
                    pS = psp.tile([128, 1024], f32, tag="s")
                    nc.tensor.matmul(
                        pS[:, 0:512],
                        lhsT=K_sb[0:64, tp, js],
                        rhs=Q_sb[0:64, tp, isl],
                        start=True,
                        stop=True,
                    )
                    nc.tensor.matmul(
                        pS[:, 512:1024],
                        lhsT=K_sb[64:128, tp, js],
                        rhs=Q_sb[64:128, tp, isl],
                        start=True,
                        stop=True,
                    )
                    Pab = pwork.tile([128, 1024], bf, tag="p")
                    nc.scalar.activation(Pab, pS, Exp, scale=SCALE)
                    pend.append((Pab, it, tp, jt))
                if len(pend) > (DEPTH if g < STEPS else 0):
                    Pab, qit, qtp, qjt = pend.pop(0)
                    qisl = slice(qit * 512, (qit + 1) * 512)
                    if qjt == 0:
                        accs[qtp] = (
                            psp.tile(
                                [D + 1, 512], f32, tag="acc", bufs=2,
                                name="pOa",
                            ),
                            psp.tile(
                                [D + 1, 512], f32, tag="acc", bufs=2,
                                name="pOb",
                            ),
                        )
                    pOa, pOb = accs[qtp]
                    ha, hb = 2 * qtp, 2 * qtp + 1
                    nc.tensor.matmul(
                        pOa,
                        lhsT=V_sb[:, qjt, ha * 65 : ha * 65 + 65],
                        rhs=Pab[:, 0:512],
                        start=(qjt == 0),
                        stop=(qjt == JT - 1),
                    )
                    nc.tensor.matmul(
                        pOb,
                        lhsT=V_sb[:, qjt, hb * 65 : hb * 65 + 65],
                        rhs=Pab[:, 512:1024],
                        start=(qjt == 0),
                        stop=(qjt == JT - 1),
                    )
                    if qjt == JT - 1:
                        # drain raw O (bf16) + denominator row to SBUF,
                        # freeing the PSUM accumulator banks for the
                        # next pair
                        for hh, pO in ((ha, pOa), (hb, pOb)):
                            oraw = normp.tile(
                                [D, 512], bf, tag="oraw", bufs=8
                            )
                            nc.vector.tensor_copy(oraw, pO[0:D, :])
                            grp = (qit % 2) * 2 + hh // 4
                            pb = 32 * (hh % 4)
                            nc.vector.tensor_copy(
                                den_sb[pb : pb + 1, grp, :],
                                pO[D : D + 1, :],
                            )
                            oraw_map[(qit, hh)] = oraw
                        del accs[qtp]
                # Wo filler slots sit after the 8 norm slots (rem 2..9) so
                # every head of i-tile it-1 is normalized in O_sb before
                # the first Wo matmul reads it
                grem = g % (NPAIR * JT)
                if (
                    g < STEPS
                    and grem >= 10
                    and (grem - 10) % 6 == 0
                    and wo_queue
                ):
                    emit_wo(wo_queue.pop(0), (g // (NPAIR * JT)) - 1)
            for hh in range(4, HPC):
                emit_norm_head(LT - 1, hh)
            for mt in wo_queue:
                emit_wo(mt, LT - 2)
            for mt in range(C // 128):
                emit_wo(mt, LT - 1)
    return nc


_NC = None


def _get_nc():
    global _NC
    if _NC is None:
        _NC = _split_drain_waits(_build_nc())
    return _NC


def _make_in_maps(query, context, Wq, Wk, Wv, Wo, bo):
    zeros_res = np.zeros((C, L), BF16)
    in_maps = []
    for c in range(NCORES):
        b, hf = c // 2, c % 2
        rows = slice(hf * CS, (hf + 1) * CS)
        in_maps.append(
            {
                "q_bf": query[b].astype(BF16),
                "ctx_bf": context[b].astype(BF16),
                "wqT": np.ascontiguousarray(Wq[rows].T).astype(BF16),
                "wkT": np.ascontiguousarray(Wk[rows].T).astype(BF16),
                "wvT": np.ascontiguousarray(Wv[rows].T).astype(BF16),
                "woT": np.ascontiguousarray(Wo[:, rows].T).astype(BF16),
                "resid": (query[b] + bo[:, None]).astype(BF16)
                if hf == 0
                else zeros_res,
            }
        )
    return in_maps


def _gather(results):
    out = np.empty((B, C, L), np.float32)
    for b in range(B):
        out[b] = results[2 * b]["out"].astype(np.float32) + results[
            2 * b + 1
        ]["out"].astype(np.float32)
    return out


def kernel(query, context, Wq, Wk, Wv, Wo, bo, heads):
    query = np.asarray(query, dtype=np.float32)
    context = np.asarray(context, dtype=np.float32)
    Wq = np.asarray(Wq, dtype=np.float32)
    Wk = np.asarray(Wk, dtype=np.float32)
    Wv = np.asarray(Wv, dtype=np.float32)
    Wo = np.asarray(Wo, dtype=np.float32)
    bo = np.asarray(bo, dtype=np.float32)
    assert int(heads) == H
    assert query.shape == (B, C, L) and context.shape == (B, C, L)

    nc = _get_nc()
    in_maps = _make_in_maps(query, context, Wq, Wk, Wv, Wo, bo)
    res = run_bass_kernel_spmd(nc, in_maps, list(range(NCORES))).results
    return _gather(res)
